# revision 31
# baseline (speedup 1.0000x reference)
"""BalanceL1Loss (hard-negative mining) on 8 Trainium2 NeuronCores.

Data-parallel over batch: each of the 8 cores gets 4 of the 32 images.

Math (matches the torch/jax reference):
    binary        = (gt > 0)
    positive      = binary * mask            -> pos_num = sum(positive)
    negative      = (1 - binary) * mask      -> neg_cnt = sum(negative)
    loss          = |pred - gt|
    pos_loss_sum  = sum(positive * loss)
    negative_num  = min(neg_cnt, 3 * pos_num)
    k             = floor(negative_num)
    neg_loss_sum  = sum of the k largest values of (negative * loss)
    out           = (pos_loss_sum + neg_loss_sum) / (pos_num + negative_num + 1e-6)
    (fallback mean(loss) when pos_num == 0)

The top-k sum is computed by threshold selection: for any t,
    f(t) = sum(relu(v - t)) + k * t
is convex in t and equals the exact top-k sum when t is the k-th largest
value of v (the count-correction term cancels:
sum_{v>t} v + (k - cnt)*t == sum(relu(v-t)) + k*t), and min over a few
candidate t is therefore an upper bound that is exact at the k-th largest.

Launch 0 (sampler) computes v and positive/negative counts on a 1/64
block-sample of the inputs on device.  The host gathers the per-core
sample lists ("all-gather the local candidate lists and reduce globally")
and picks 2 candidate thresholds around the estimated k-th largest rank.
The main launch then streams the full inputs exactly once (DMA-bound):
all scalar reductions ride on the compute instructions via accum_out, the
mask / mask*loss sums use ones-matmuls on the otherwise-idle TensorEngine,
and the relu(v - t_c) accumulations run on ScalarE — v never leaves
the chip.  The candidate thresholds only pivot the evaluation; every term
of the answer (including the exact k) comes from full-data device sums, so
sampling error only perturbs f() quadratically around its minimum.  The
host reduces per-core partials in float64 and takes min over candidates.

Infra note: the walrus in this container accepts at most one sem-wait per
instruction while this concourse's TileContext packs several — see
_split_multiwait_bir.
"""

import numpy as np
from contextlib import ExitStack

# ---- problem geometry (hardcoded per contest rules) ----
B, H, W = 32, 768, 768
NCORES = 8
B_LOCAL = B // NCORES              # 4 images per core
P = 128                            # SBUF partitions
N_TOTAL = B * H * W                # 18_874_368
N_LOCAL = B_LOCAL * H * W          # 2_359_296
FREE = N_LOCAL // P                # 18432
TILE_F = 2048                      # free elems per tile
NT = FREE // TILE_F                # 9 tiles
SSTRIDE = 64                       # sample rate 1/64 (blocks of 32 per 2048)
SBLOCK = TILE_F // SSTRIDE         # 32-wide sample block per 2048 columns
NS = FREE // SSTRIDE               # 288 sampled columns / partition
NCAND = 2                          # threshold candidates in the main launch
NEG_RATIO = 3.0
# uniform tile widths modeled best (head/tail splits gained nothing: the
# startup gap is fixed dispatch preamble, not first-tile DMA size)
MAIN_WIDTHS = [TILE_F] * NT
NTM = len(MAIN_WIDTHS)

_CACHE = {}


def _split_multiwait_bir(bir_bytes):
    """Walrus in this container accepts at most ONE sem-wait per instruction
    (CoreV3GenImpl setupSyncWait: 'Too many sync wait commands'), while
    TileContext packs several.  Hoist all but the last wait of every
    instruction onto fresh same-engine NoOps placed directly before it —
    semantically identical (sem counters are monotone)."""
    import json
    bir = json.loads(bir_bytes)
    n = 0
    for fn in bir["functions"]:
        for blk in fn["blocks"]:
            out = []
            for inst in blk["instructions"]:
                si = inst.get("sync_info")
                ow = (si or {}).get("on_wait") or []
                if len(ow) > 1:
                    for w in ow[:-1]:
                        n += 1
                        out.append({
                            "debug": inst.get("debug"),
                            "engine": inst["engine"],
                            "ins": [],
                            "name": f"I-wsplit{n}",
                            "opcode": "NoOp",
                            "outs": [],
                            "text_hint": "wait_split",
                            "sync_info": {"on_wait": [w], "on_update": []},
                        })
                    si["on_wait"] = [ow[-1]]
                out.append(inst)
            blk["instructions"] = out
    return json.dumps(bir).encode()


def _patch_bass():
    import concourse.bass as bass
    if getattr(bass.Bass, "_wsplit_patched", False):
        return
    orig = bass.Bass.to_json_bytes

    def to_json_bytes(self):
        return _split_multiwait_bir(orig(self))

    bass.Bass.to_json_bytes = to_json_bytes
    bass.Bass._wsplit_patched = True


def _bass_mods():
    import concourse.bass as bass
    import concourse.tile as tile
    from concourse import mybir
    _patch_bass()
    return bass, tile, mybir


def build_l0(ns=NS):
    """Sampling pre-pass over host-sliced 1/64 input blocks.

    inputs : ps, gs, ms  [P, ns] f32  (block-sampled pred/gt/mask columns)
    outputs: samples     [P, ns] bf16 (sampled negative*loss values)
             acc0        [P, 2] f32   (sum(negative), sum(positive) samples)
    """
    bass, tile, mybir = _bass_mods()
    f32, bf16 = mybir.dt.float32, mybir.dt.bfloat16
    A = mybir.AluOpType
    AF = mybir.ActivationFunctionType

    nc = bass.Bass("TRN2", target_bir_lowering=False, debug=False)
    ps = nc.dram_tensor("ps", [P, ns], f32, kind="ExternalInput").ap()
    gs = nc.dram_tensor("gs", [P, ns], f32, kind="ExternalInput").ap()
    ms = nc.dram_tensor("ms", [P, ns], f32, kind="ExternalInput").ap()
    samples = nc.dram_tensor("samples", [P, ns], bf16, kind="ExternalOutput").ap()
    acc0 = nc.dram_tensor("acc0", [P, 2], f32, kind="ExternalOutput").ap()

    with tile.TileContext(nc) as tc, ExitStack() as ctx:
        pool = ctx.enter_context(tc.tile_pool(name="pool", bufs=1))
        tP = pool.tile([P, ns], f32)
        nc.sync.dma_start(out=tP[:], in_=ps[:])
        tG = pool.tile([P, ns], f32)
        nc.sync.dma_start(out=tG[:], in_=gs[:])
        tM = pool.tile([P, ns], f32)
        nc.sync.dma_start(out=tM[:], in_=ms[:])
        acc_sb = pool.tile([P, 2], f32)

        diff = pool.tile([P, ns], bf16)
        nc.vector.tensor_tensor(diff[:], tP[:], tG[:], A.subtract)
        lossb = pool.tile([P, ns], bf16)
        nc.scalar.activation(lossb[:], diff[:], AF.Abs)
        nmb = pool.tile([P, ns], bf16)
        nc.vector.scalar_tensor_tensor(nmb[:], tG[:], 0.0, tM[:],
                                       A.is_le, A.mult,
                                       accum_out=acc_sb[:, 0:1])
        pmb = pool.tile([P, ns], bf16)
        nc.vector.scalar_tensor_tensor(pmb[:], tG[:], 0.0, tM[:],
                                       A.is_gt, A.mult,
                                       accum_out=acc_sb[:, 1:2])
        nv = pool.tile([P, ns], bf16)
        nc.vector.scalar_tensor_tensor(nv[:], nmb[:], 0.0, lossb[:],
                                       A.bypass, A.mult)
        nc.sync.dma_start(out=samples[:], in_=nv[:])
        nc.sync.dma_start(out=acc0[:], in_=acc_sb[:])
    return nc


def build_main(free=FREE, tile_f=TILE_F, ncand=NCAND, widths=None):
    """Fused full pass — streams the inputs exactly once, nothing O(N) leaves
    the chip.

    `widths` is the tile-width schedule (512-multiples summing to `free`);
    narrow head tiles let the engines start ~4x sooner than a full 2048-wide
    first tile, narrow tail tiles shorten the final dependency chain.

    inputs : pred, gt, mask        [P, free] f32
             tneg                  [P, ncand] f32  (-t_c relu biases)
    outputs: acc  [P, (3+ncand)*nt] f32  per-tile free-dim sums (nt tiles):
                 cols [0,nt)    sum(negative)           (nm)
                 cols [nt,2nt)  sum(negative * loss)    (negv)
                 cols [2nt,3nt) sum(loss)               (fallback path)
                 cols [(3+c)*nt, (4+c)*nt) sum(relu(negative*loss - t_c))
             sums [2, 512] f32  TensorE column sums: row0 mask, row1 mask*loss
    """
    bass, tile, mybir = _bass_mods()
    f32, bf16 = mybir.dt.float32, mybir.dt.bfloat16
    A = mybir.AluOpType
    AF = mybir.ActivationFunctionType

    if widths is None:
        widths = [tile_f] * (free // tile_f)
    assert sum(widths) == free and all(w % 512 == 0 for w in widths)
    nt = len(widths)

    nc = bass.Bass("TRN2", target_bir_lowering=False, debug=False)
    pred = nc.dram_tensor("pred", [P, free], f32, kind="ExternalInput").ap()
    gt = nc.dram_tensor("gt", [P, free], f32, kind="ExternalInput").ap()
    mask = nc.dram_tensor("mask", [P, free], f32, kind="ExternalInput").ap()
    tneg = nc.dram_tensor("tneg", [P, ncand], f32, kind="ExternalInput").ap()
    acc = nc.dram_tensor("acc", [P, (3 + ncand) * nt], f32,
                         kind="ExternalOutput").ap()
    sums = nc.dram_tensor("sums", [2, 512], f32, kind="ExternalOutput").ap()

    with tile.TileContext(nc) as tc, ExitStack() as ctx:
        io = ctx.enter_context(tc.tile_pool(name="io", bufs=3))
        mid = ctx.enter_context(tc.tile_pool(name="mid", bufs=3))
        st = ctx.enter_context(tc.tile_pool(name="st", bufs=1))
        ps = ctx.enter_context(tc.tile_pool(name="ps", bufs=1, space="PSUM"))

        ones = st.tile([P, 1], bf16)
        nc.vector.memset(ones[:], 1.0)
        tn = st.tile([P, ncand], f32)
        nc.sync.dma_start(out=tn[:], in_=tneg[:])
        acc_sb = st.tile([P, (3 + ncand) * nt], f32)
        msum_ps = ps.tile([1, 512], f32, tag="msum_ps")
        mlsum_ps = ps.tile([1, 512], f32, tag="mlsum_ps")

        off = 0
        for j, w in enumerate(widths):
            s = bass.ds(off, w)
            tP = io.tile([P, w], f32, tag="tP")
            nc.sync.dma_start(out=tP[:], in_=pred[:, s])
            tG = io.tile([P, w], f32, tag="tG")
            nc.sync.dma_start(out=tG[:], in_=gt[:, s])
            tM = io.tile([P, w], f32, tag="tM")
            nc.sync.dma_start(out=tM[:], in_=mask[:, s])

            diff = mid.tile([P, w], bf16, tag="diff")
            nc.vector.tensor_tensor(diff[:], tP[:], tG[:], A.subtract)

            lossb = mid.tile([P, w], bf16, tag="lossb")
            nc.scalar.activation(lossb[:], diff[:], AF.Abs,
                                 accum_out=acc_sb[:, 2 * nt + j:2 * nt + j + 1])

            maskb = mid.tile([P, w], bf16, tag="maskb")
            nc.scalar.activation(maskb[:], tM[:], AF.Copy)

            nmb = mid.tile([P, w], bf16, tag="nmb")
            nc.vector.scalar_tensor_tensor(nmb[:], tG[:], 0.0, tM[:],
                                           A.is_le, A.mult,
                                           accum_out=acc_sb[:, j:j + 1])

            nv = mid.tile([P, w], bf16, tag="nv")
            nc.vector.scalar_tensor_tensor(nv[:], nmb[:], 0.0, lossb[:],
                                           A.bypass, A.mult,
                                           accum_out=acc_sb[:, nt + j:nt + j + 1])

            mlb = mid.tile([P, w], bf16, tag="mlb")
            nc.vector.tensor_tensor(mlb[:], maskb[:], lossb[:], A.mult)

            for c in range(ncand):
                dummy = mid.tile([P, w], bf16, tag="relud")
                nc.scalar.activation(dummy[:], nv[:], AF.Relu,
                                     bias=tn[:, c:c + 1],
                                     accum_out=acc_sb[:, (3 + c) * nt + j:
                                                      (3 + c) * nt + j + 1])

            for c in range(w // 512):
                cs = bass.ts(c, 512)
                first = (j == 0 and c == 0)
                last = (j == nt - 1 and c == w // 512 - 1)
                nc.tensor.matmul(msum_ps[:], ones[:], maskb[:, cs],
                                 start=first, stop=last)
                nc.tensor.matmul(mlsum_ps[:], ones[:], mlb[:, cs],
                                 start=first, stop=last)
            off += w

        for row, src in enumerate((msum_ps, mlsum_ps)):
            row_sb = st.tile([1, 512], f32, tag=f"row_sb{row}")
            nc.vector.tensor_copy(row_sb[:], src[:])
            nc.sync.dma_start(out=sums[row:row + 1, :], in_=row_sb[:])
        nc.sync.dma_start(out=acc[:], in_=acc_sb[:])
    return nc


def _get_programs():
    if "l0" not in _CACHE:
        _CACHE["l0"] = build_l0()
        _CACHE["main"] = build_main(widths=MAIN_WIDTHS)
    return _CACHE["l0"], _CACHE["main"]


def _run_spmd(nc, in_maps, **kw):
    from concourse.bass_utils import run_bass_kernel_spmd
    return run_bass_kernel_spmd(nc, in_maps, list(range(NCORES)), **kw)


def kernel(pred, gt, mask):
    pred = np.ascontiguousarray(np.asarray(pred, dtype=np.float32))
    gt = np.ascontiguousarray(np.asarray(gt, dtype=np.float32))
    mask = np.ascontiguousarray(np.asarray(mask, dtype=np.float32))
    assert pred.shape == (B, H, W), pred.shape

    l0, main = _get_programs()

    def core_view(x, c):
        return x[c * B_LOCAL:(c + 1) * B_LOCAL].reshape(P, FREE)

    def sample_blocks(x, c):
        # 32 contiguous columns out of every 2048 (rate exactly 1/64)
        v = core_view(x, c).reshape(P, NT, TILE_F)[:, :, :SBLOCK]
        return np.ascontiguousarray(v).reshape(P, NS)

    # ---- launch 0: sampled negative-loss values + count estimates ----
    in_maps0 = [{"ps": sample_blocks(pred, c),
                 "gs": sample_blocks(gt, c),
                 "ms": sample_blocks(mask, c)} for c in range(NCORES)]
    res0 = _run_spmd(l0, in_maps0).results

    nm_s = sum(r["acc0"][:, 0].astype(np.float64).sum() for r in res0)
    pm_s = sum(r["acc0"][:, 1].astype(np.float64).sum() for r in res0)
    s = np.concatenate([r["samples"].reshape(-1) for r in res0]).astype(np.float32)
    S = s.size

    k_est = int(np.floor(min(nm_s * SSTRIDE, NEG_RATIO * pm_s * SSTRIDE)))
    m_rank = int(np.clip(round(k_est / SSTRIDE), 1, S))
    dm = max(1, int(2.5 * np.sqrt(m_rank) + 0.01 * m_rank))
    cands = []
    for mm in (m_rank, m_rank - dm, m_rank + dm):
        mm = int(np.clip(mm, 1, S))
        cands.append(max(float(np.partition(s, S - mm)[S - mm]), 0.0))
    cands = (cands + cands[-1:] * NCAND)[:NCAND]

    # ---- main launch: full-data sums + relu(v - t_c) sums, v stays on chip
    tneg = np.zeros((P, NCAND), dtype=np.float32)
    for ci, t in enumerate(cands):
        tneg[:, ci] = -t
    in_maps = [{"pred": core_view(pred, c),
                "gt": core_view(gt, c),
                "mask": core_view(mask, c),
                "tneg": tneg} for c in range(NCORES)]
    res = _run_spmd(main, in_maps).results

    # ---- combine per-core partials (exact, float64) ----
    nm_sum = 0.0
    negv_sum = 0.0
    loss_sum = 0.0
    mask_sum = 0.0
    ml_sum = 0.0
    relu_sums = [0.0] * NCAND
    for c in range(NCORES):
        a = res[c]["acc"].astype(np.float64)
        nm_sum += a[:, 0:NTM].sum()
        negv_sum += a[:, NTM:2 * NTM].sum()
        loss_sum += a[:, 2 * NTM:3 * NTM].sum()
        for ci in range(NCAND):
            relu_sums[ci] += a[:, (3 + ci) * NTM:(4 + ci) * NTM].sum()
        su = res[c]["sums"].astype(np.float64)
        mask_sum += su[0].sum()
        ml_sum += su[1].sum()

    pos_num = mask_sum - nm_sum
    neg_cnt = nm_sum
    pos_loss = ml_sum - negv_sum

    if pos_num == 0.0:
        return np.asarray(loss_sum / N_TOTAL, dtype=np.float32)

    negative_num = min(neg_cnt, NEG_RATIO * pos_num)
    k = int(np.floor(negative_num))

    if k <= 0:
        neg_loss = 0.0
    else:
        neg_loss = min(relu_sums[ci] + k * t for ci, t in enumerate(cands))

    balance = (pos_loss + neg_loss) / (pos_num + negative_num + 1e-6)
    return np.asarray(balance, dtype=np.float32)
